# revision 4
# baseline (speedup 1.0000x reference)
"""Multi-head attention (B=2, N=2048, D=768, H=12, Dh=64) on 8 TRN2 NeuronCores.

Sharding: head-parallel Megatron-style. Core c handles batch b=c//4 and heads
[3*(c%4), 3*(c%4)+3). Each core projects q/k/v for its 3 heads (column-sliced
Wq/Wkv), runs softmax(q k^T/8) v on-chip, and computes a partial out-projection
against its row-slice of Wproj. Host sums the 4 partials per batch + bias.

On-chip layout: sources are host-pre-transposed so projections are natural
matmuls. Scores are computed transposed (S^T: k on partitions, q free) so the
attn@v matmul consumes exp(S^T) directly as the streaming operand with
lhsT = [v | ones]; the ones column yields the softmax denominator for free.
"""

import os
import sys

sys.path.insert(0, "/opt/trn_rl_repo")

from contextlib import ExitStack

import ml_dtypes
import numpy as np

import concourse.bass as bass
import concourse.bacc as bacc
import concourse.tile as tile
from concourse import mybir
from concourse.bass_utils import run_bass_kernel_spmd

bf16 = ml_dtypes.bfloat16
F32 = mybir.dt.float32
BF16 = mybir.dt.bfloat16
EXP = mybir.ActivationFunctionType.Exp

P = 128          # partitions
NQ = 2048        # query length (per batch)
NKV = 2048       # kv length
D = 768          # model dim
DH = 64          # head dim
HL = 3           # heads per core
DL = HL * DH     # local projected dim (192)
KB = D // P      # contraction blocks for projections (6)
NKB = NKV // P   # k-index blocks (16)
QC = 1024        # q chunk for the attention inner loop
NQC = NQ // QC   # 2
SCALE = DH ** -0.5

_CACHE: dict = {}
LAST_RESULTS = None


def _build_program() -> bass.Bass:
    nc = bacc.Bacc("TRN2", target_bir_lowering=False)

    qsT = nc.dram_tensor("qsT", [D, NQ], BF16, kind="ExternalInput")
    kvT = nc.dram_tensor("kvT", [D, NKV], BF16, kind="ExternalInput")
    wq = nc.dram_tensor("wq", [D, DL], BF16, kind="ExternalInput")
    wk = nc.dram_tensor("wk", [D, DL], BF16, kind="ExternalInput")
    wv = nc.dram_tensor("wv", [D, DL], BF16, kind="ExternalInput")
    wp = nc.dram_tensor("wp", [DL, D], BF16, kind="ExternalInput")
    out = nc.dram_tensor("out", [NQ, D], BF16, kind="ExternalOutput")

    with tile.TileContext(nc) as tc, ExitStack() as ctx:
        sb_src = ctx.enter_context(tc.tile_pool(name="src", bufs=KB))
        sb_w = ctx.enter_context(tc.tile_pool(name="wts", bufs=KB))
        sb_p = ctx.enter_context(tc.tile_pool(name="persist", bufs=1))
        sb_es = ctx.enter_context(tc.tile_pool(name="es", bufs=4))
        sb_sm = ctx.enter_context(tc.tile_pool(name="small", bufs=2))
        sb_ob = ctx.enter_context(tc.tile_pool(name="outsb", bufs=3))

        # ---- DMA inputs ----
        qsT_sb, kvT_sb, wq_sb, wk_sb, wv_sb = [], [], [], [], []
        for kb in range(KB):
            t = sb_src.tile([P, NQ], BF16, tag="qsT")
            nc.sync.dma_start(t[:], qsT[kb * P : (kb + 1) * P, :])
            qsT_sb.append(t)
            t = sb_src.tile([P, NKV], BF16, tag="kvT")
            nc.sync.dma_start(t[:], kvT[kb * P : (kb + 1) * P, :])
            kvT_sb.append(t)
            for lst, dram, tag in ((wq_sb, wq, "wq"), (wk_sb, wk, "wk"), (wv_sb, wv, "wv")):
                t = sb_w.tile([P, DL], BF16, tag=tag)
                nc.sync.dma_start(t[:], dram[kb * P : (kb + 1) * P, :])
                lst.append(t)
        wp01 = sb_p.tile([P, D], BF16, tag="wp01")
        nc.sync.dma_start(wp01[:], wp[0:P, :])
        wp2 = sb_p.tile([DH, D], BF16, tag="wp2")
        nc.sync.dma_start(wp2[:], wp[P : P + DH, :])

        # ---- persistent intermediates ----
        qT01 = sb_p.tile([P, NQ], BF16, tag="qT01")   # q^T heads 0,1 (d on partitions)
        kT01 = sb_p.tile([P, NKV], BF16, tag="kT01")
        qT2 = sb_p.tile([DH, NQ], BF16, tag="qT2")    # q^T head 2
        kT2 = sb_p.tile([DH, NKV], BF16, tag="kT2")
        vA = sb_p.tile([P, HL * NKB * 65], BF16, tag="vA")  # per (h, kb): [v(64) | ones]
        X01 = sb_p.tile([P, NQ], BF16, tag="X01")     # normalized x^T heads 0,1
        X2 = sb_p.tile([DH, NQ], BF16, tag="X2")
        nc.vector.memset(vA[:], 1.0)  # ones columns; v evacs overwrite the rest

        # Two kernel-lifetime PSUM pools (no phase barriers):
        #   psA (2 slots x 2 banks): projection chains, scores, out-proj
        #   psB (2 slots x 2 banks): v-proj chains, attn@v accumulators
        psA = ctx.enter_context(tc.tile_pool(name="psA", bufs=2, space="PSUM"))
        psB = ctx.enter_context(tc.tile_pool(name="psB", bufs=2, space="PSUM"))

        # ================= Phase 1: projections =================
        # q^T / k^T for heads 0,1 (full 128-column weight blocks)
        for w_sb, src_sb, dst in ((wq_sb, qsT_sb, qT01), (wk_sb, kvT_sb, kT01)):
            for half in range(NQC):
                ps = psA.tile([P, QC], F32, tag="A")
                for kb in range(KB):
                    for j in range(QC // 512):
                        nc.tensor.matmul(
                            ps[:, j * 512 : (j + 1) * 512],
                            w_sb[kb][:, 0:P],
                            src_sb[kb][:, half * QC + j * 512 : half * QC + (j + 1) * 512],
                            start=(kb == 0),
                            stop=(kb == KB - 1),
                        )
                nc.scalar.copy(dst[:, half * QC : (half + 1) * QC], ps[:])
        # q^T / k^T head 2 — col-tiled pair (q at psum 0:64, k at 64:128)
        for half in range(NQC):
            ps = psA.tile([P, QC], F32, tag="A")
            for kb in range(KB):
                for j in range(QC // 512):
                    sl = slice(j * 512, (j + 1) * 512)
                    src_sl = slice(half * QC + j * 512, half * QC + (j + 1) * 512)
                    nc.tensor.matmul(
                        ps[0:DH, sl], wq_sb[kb][:, P:DL], qsT_sb[kb][:, src_sl],
                        start=(kb == 0), stop=(kb == KB - 1),
                    )
                    nc.tensor.matmul(
                        ps[DH:P, sl], wk_sb[kb][:, P:DL], kvT_sb[kb][:, src_sl],
                        start=(kb == 0), stop=(kb == KB - 1),
                    )
            nc.scalar.copy(qT2[:, half * QC : (half + 1) * QC], ps[0:DH, :])
            nc.scalar.copy(kT2[:, half * QC : (half + 1) * QC], ps[DH:P, :])
        # v projection: (k-idx, d_local), scattered into vA with ones columns
        vA_view = vA[:].rearrange("p (h k c) -> p h k c", h=HL, k=NKB)
        for m in range(NKB):
            ps = psB.tile([P, DL], F32, tag="B")
            for kb in range(KB):
                nc.tensor.matmul(
                    ps[:], kvT_sb[kb][:, m * P : (m + 1) * P], wv_sb[kb][:],
                    start=(kb == 0), stop=(kb == KB - 1),
                )
            nc.scalar.copy(
                vA_view[:, :, m, 0:DH],
                ps[:].rearrange("p (h d) -> p h d", h=HL),
            )

        # ====== Phase 2+3 interleaved: attention per q-half, then out-proj for that half ======
        for qc in range(NQC):
            for h in range(HL):
                if h < 2:
                    kT_h = kT01[h * DH : (h + 1) * DH, :]
                    qT_h = qT01[h * DH : (h + 1) * DH, :]
                    X_h = X01[h * DH : (h + 1) * DH, :]
                else:
                    kT_h, qT_h, X_h = kT2[:], qT2[:], X2[:]
                xps = psB.tile([65, QC], F32, tag="B")
                for kb in range(NKB):
                    sc = psA.tile([P, QC], F32, tag="A")
                    for j in range(QC // 512):
                        nc.tensor.matmul(
                            sc[:, j * 512 : (j + 1) * 512],
                            kT_h[:, kb * P : (kb + 1) * P],
                            qT_h[:, qc * QC + j * 512 : qc * QC + (j + 1) * 512],
                            start=True, stop=True,
                        )
                    es = sb_es.tile([P, QC], BF16, tag="es")
                    nc.scalar.activation(es[:], sc[:], EXP, scale=SCALE)
                    for j in range(QC // 512):
                        sl = slice(j * 512, (j + 1) * 512)
                        nc.tensor.matmul(
                            xps[:, sl],
                            vA[:, (h * NKB + kb) * 65 : (h * NKB + kb + 1) * 65],
                            es[:, sl],
                            start=(kb == 0), stop=(kb == NKB - 1),
                        )
                # softmax denominator -> reciprocal -> broadcast -> normalize
                dn = sb_sm.tile([1, QC], F32, tag="dn")
                nc.vector.tensor_copy(dn[:], xps[64:65, :])
                rc = sb_sm.tile([1, QC], F32, tag="rc")
                nc.vector.reciprocal_approx_fast(rc[:], dn[:])
                rcb = sb_sm.tile([1, QC], BF16, tag="rcb")
                nc.vector.tensor_copy(rcb[:], rc[:])
                bcs = sb_sm.tile([DH, QC], BF16, tag="bcs")
                nc.gpsimd.partition_broadcast(bcs[:], rcb[:])
                nc.vector.tensor_mul(
                    X_h[:, qc * QC : (qc + 1) * QC], xps[0:DH, :], bcs[:]
                )
            # out-projection for the q-tiles of this half
            for m in range(qc * (NKB // NQC), (qc + 1) * (NKB // NQC)):
                po = psA.tile([P, D], F32, tag="A")
                for j, n in ((0, 512), (512, 256)):
                    nc.tensor.matmul(
                        po[:, j : j + n], X01[:, m * P : (m + 1) * P], wp01[:, j : j + n],
                        start=True, stop=False,
                    )
                    nc.tensor.matmul(
                        po[:, j : j + n], X2[:, m * P : (m + 1) * P], wp2[:, j : j + n],
                        start=False, stop=True,
                    )
                ob = sb_ob.tile([P, D], BF16, tag="ob")
                nc.vector.tensor_copy(ob[:], po[:])
                nc.sync.dma_start(out[m * P : (m + 1) * P, :], ob[:])

    nc.compile()
    return nc


def _get_nc() -> bass.Bass:
    if "nc" not in _CACHE:
        _CACHE["nc"] = _build_program()
    return _CACHE["nc"]


def kernel(**inputs) -> np.ndarray:
    global LAST_RESULTS
    qs = np.asarray(inputs["query_source"], dtype=np.float32)
    kv = np.asarray(inputs["kv_source"], dtype=np.float32)
    Wq = np.asarray(inputs["Wq"], dtype=np.float32)
    Wkv = np.asarray(inputs["Wkv"], dtype=np.float32)
    Wp = np.asarray(inputs["Wproj"], dtype=np.float32)
    bp = np.asarray(inputs["bproj"], dtype=np.float32)

    nc = _get_nc()
    in_maps = []
    for c in range(8):
        b = c // 4
        c0 = (c % 4) * DL
        in_maps.append(
            {
                "qsT": np.ascontiguousarray(qs[b].T).astype(bf16),
                "kvT": np.ascontiguousarray(kv[b].T).astype(bf16),
                "wq": Wq[:, c0 : c0 + DL].astype(bf16),
                "wk": Wkv[:, c0 : c0 + DL].astype(bf16),
                "wv": Wkv[:, D + c0 : D + c0 + DL].astype(bf16),
                "wp": Wp[c0 : c0 + DL, :].astype(bf16),
            }
        )

    trace = bool(int(os.environ.get("KERNEL_TRACE", "0")))
    res = run_bass_kernel_spmd(nc, in_maps, list(range(8)), trace=trace)
    LAST_RESULTS = res

    out = np.tile(bp.astype(np.float32), (2, NQ, 1))
    for c in range(8):
        out[c // 4] += res.results[c]["out"].astype(np.float32)
    return out


# revision 8
# speedup vs baseline: 1.0898x; 1.0898x over previous
"""Multi-head attention (B=2, N=2048, D=768, H=12, Dh=64) on 8 TRN2 NeuronCores.

Sharding: head-parallel Megatron-style. Core c handles batch b=c//4 and heads
[3*(c%4), 3*(c%4)+3). Each core projects q/k/v for its 3 heads (column-sliced
Wq/Wkv), runs softmax(q k^T/8) v on-chip, and computes a partial out-projection
against its row-slice of Wproj. Host sums the 4 partials per batch + bias.

On-chip layout: sources are host-pre-transposed so projections are natural
matmuls. Scores are computed transposed (S^T: k on partitions, q free) so the
attn@v matmul consumes exp(S^T) directly as the streaming operand with
lhsT = [v | ones]; the ones column yields the softmax denominator for free.
"""

import os
import sys

sys.path.insert(0, "/opt/trn_rl_repo")

from contextlib import ExitStack

import ml_dtypes
import numpy as np

import concourse.bass as bass
import concourse.bacc as bacc
import concourse.tile as tile
from concourse import mybir
from concourse.bass_utils import run_bass_kernel_spmd

bf16 = ml_dtypes.bfloat16
F32 = mybir.dt.float32
BF16 = mybir.dt.bfloat16
EXP = mybir.ActivationFunctionType.Exp

P = 128          # partitions
NQ = 2048        # query length (per batch)
NKV = 2048       # kv length
D = 768          # model dim
DH = 64          # head dim
HL = 3           # heads per core
DL = HL * DH     # local projected dim (192)
KB = D // P      # contraction blocks for projections (6)
NKB = NKV // P   # k-index blocks (16)
QC = 1024        # q chunk for the attention inner loop
NQC = NQ // QC   # 2
SCALE = DH ** -0.5

_CACHE: dict = {}
LAST_RESULTS = None


def _build_program() -> bass.Bass:
    nc = bacc.Bacc("TRN2", target_bir_lowering=False)

    qsT = nc.dram_tensor("qsT", [D, NQ], BF16, kind="ExternalInput")
    kvT = nc.dram_tensor("kvT", [D, NKV], BF16, kind="ExternalInput")
    wq = nc.dram_tensor("wq", [D, DL], BF16, kind="ExternalInput")
    wk = nc.dram_tensor("wk", [D, DL], BF16, kind="ExternalInput")
    wv = nc.dram_tensor("wv", [D, DL], BF16, kind="ExternalInput")
    wp = nc.dram_tensor("wp", [DL, D], BF16, kind="ExternalInput")
    out = nc.dram_tensor("out", [NQ, D], BF16, kind="ExternalOutput")

    with tile.TileContext(nc) as tc, ExitStack() as ctx:
        sb_src = ctx.enter_context(tc.tile_pool(name="src", bufs=KB))
        sb_w = ctx.enter_context(tc.tile_pool(name="wts", bufs=KB))
        sb_p = ctx.enter_context(tc.tile_pool(name="persist", bufs=1))
        sb_es = ctx.enter_context(tc.tile_pool(name="es", bufs=4))
        sb_sm = ctx.enter_context(tc.tile_pool(name="small", bufs=2))
        sb_ob = ctx.enter_context(tc.tile_pool(name="outsb", bufs=3))

        # ---- DMA inputs ----
        qsT_sb, kvT_sb, wq_sb, wk_sb, wv_sb = [], [], [], [], []
        for kb in range(KB):
            t = sb_src.tile([P, NKV], BF16, tag="kvT")
            nc.sync.dma_start(t[:], kvT[kb * P : (kb + 1) * P, :])
            kvT_sb.append(t)
            for lst, dram, tag in ((wk_sb, wk, "wk"), (wv_sb, wv, "wv")):
                t = sb_w.tile([P, DL], BF16, tag=tag)
                nc.sync.dma_start(t[:], dram[kb * P : (kb + 1) * P, :])
                lst.append(t)
        for kb in range(KB):
            t = sb_src.tile([P, NQ], BF16, tag="qsT")
            nc.sync.dma_start(t[:], qsT[kb * P : (kb + 1) * P, :])
            qsT_sb.append(t)
            t = sb_w.tile([P, DL], BF16, tag="wq")
            nc.sync.dma_start(t[:], wq[kb * P : (kb + 1) * P, :])
            wq_sb.append(t)
        wp01 = sb_p.tile([P, D], BF16, tag="wp01")
        nc.sync.dma_start(wp01[:], wp[0:P, :])
        wp2 = sb_p.tile([DH, D], BF16, tag="wp2")
        nc.sync.dma_start(wp2[:], wp[P : P + DH, :])

        # ---- persistent intermediates ----
        qT01 = sb_p.tile([P, NQ], BF16, tag="qT01")   # q^T heads 0,1 (d on partitions)
        kT01 = sb_p.tile([P, NKV], BF16, tag="kT01")
        qT2 = sb_p.tile([DH, NQ], BF16, tag="qT2")    # q^T head 2
        kT2 = sb_p.tile([DH, NKV], BF16, tag="kT2")
        vA = sb_p.tile([P, HL * NKB * 65], BF16, tag="vA")  # per (h, kb): [v(64) | ones]
        X01 = sb_p.tile([P, NQ], BF16, tag="X01")     # normalized x^T heads 0,1
        X2 = sb_p.tile([DH, NQ], BF16, tag="X2")
        nc.vector.memset(vA[:], 1.0)  # ones columns; v evacs overwrite the rest

        # Two kernel-lifetime PSUM pools (no phase barriers):
        #   psA (2 slots x 2 banks): scores + kT01/qT01-h0 startup chains
        #   psB (2 slots x 2 banks): v-proj, attn@v accumulators, dripped fillers
        psA = ctx.enter_context(tc.tile_pool(name="psA", bufs=2, space="PSUM"))
        psB = ctx.enter_context(tc.tile_pool(name="psB", bufs=2, space="PSUM"))

        def proj01_chain(pool, w_sb, src_sb, dst, half, evac):
            """(128,1024) chain: dst[:, half] = (w block cols 0:128).T @ srcT."""
            ps = pool.tile([P, QC], F32, tag="A" if pool is psA else "B")
            for kb in range(KB):
                for j in range(QC // 512):
                    nc.tensor.matmul(
                        ps[:, j * 512 : (j + 1) * 512],
                        w_sb[kb][:, 0:P],
                        src_sb[kb][:, half * QC + j * 512 : half * QC + (j + 1) * 512],
                        start=(kb == 0),
                        stop=(kb == KB - 1),
                    )
            evac(dst[:, half * QC : (half + 1) * QC], ps[:])

        def proj2_chain(pool, half):
            """col-tiled pair: qT2 (psum 0:64) / kT2 (psum 64:128) for one q-half."""
            ps = pool.tile([P, QC], F32, tag="A" if pool is psA else "B")
            for kb in range(KB):
                for j in range(QC // 512):
                    sl = slice(j * 512, (j + 1) * 512)
                    src_sl = slice(half * QC + j * 512, half * QC + (j + 1) * 512)
                    nc.tensor.matmul(
                        ps[0:DH, sl], wq_sb[kb][:, P:DL], qsT_sb[kb][:, src_sl],
                        start=(kb == 0), stop=(kb == KB - 1),
                    )
                    nc.tensor.matmul(
                        ps[DH:P, sl], wk_sb[kb][:, P:DL], kvT_sb[kb][:, src_sl],
                        start=(kb == 0), stop=(kb == KB - 1),
                    )
            nc.vector.tensor_copy(qT2[:, half * QC : (half + 1) * QC], ps[0:DH, :])
            nc.vector.tensor_copy(kT2[:, half * QC : (half + 1) * QC], ps[DH:P, :])

        vA_view = vA[:].rearrange("p (h k c) -> p h k c", h=HL, k=NKB)

        def v_chain(m):
            ps = psB.tile([P, DL], F32, tag="B")
            for kb in range(KB):
                nc.tensor.matmul(
                    ps[:], kvT_sb[kb][:, m * P : (m + 1) * P], wv_sb[kb][:],
                    start=(kb == 0), stop=(kb == KB - 1),
                )
            nc.vector.tensor_copy(
                vA_view[:, :, m, 0:DH],
                ps[:].rearrange("p (h d) -> p h d", h=HL),
            )

        def outproj_tile(m, pool, evac=None):
            po = pool.tile([P, D], F32, tag="A" if pool is psA else "B")
            for j, n in ((0, 512), (512, 256)):
                nc.tensor.matmul(
                    po[:, j : j + n], X01[:, m * P : (m + 1) * P], wp01[:, j : j + n],
                    start=True, stop=False,
                )
                nc.tensor.matmul(
                    po[:, j : j + n], X2[:, m * P : (m + 1) * P], wp2[:, j : j + n],
                    start=False, stop=True,
                )
            ob = sb_ob.tile([P, D], BF16, tag="ob")
            (evac or nc.vector.tensor_copy)(ob[:], po[:])
            nc.sync.dma_start(out[m * P : (m + 1) * P, :], ob[:])

        # ---- startup: the minimum needed for head-0 / q-half-0 scores ----
        proj01_chain(psA, wk_sb, kvT_sb, kT01, 0, nc.scalar.copy)   # scores kb 0..7
        proj01_chain(psA, wq_sb, qsT_sb, qT01, 0, nc.scalar.copy)   # q-half 0
        v_chain(0)

        # Fillers dripped into the attention kb-loops (emitted at given kb index).
        f00 = {kb: [(lambda m=kb + 1: v_chain(m))] for kb in range(0, 15)}
        f00[3] = f00[3] + [lambda: proj01_chain(psB, wk_sb, kvT_sb, kT01, 1, nc.scalar.copy)]
        fillers = {
            (0, 0): f00,
            (0, 1): {
                2: [lambda: proj01_chain(psB, wq_sb, qsT_sb, qT01, 1, nc.vector.tensor_copy)],
                8: [lambda: proj2_chain(psB, 0)],
            },
            (0, 2): {1: [lambda: proj2_chain(psB, 1)]},
            (1, 0): {kb: [(lambda m=kb // 2 - 1: outproj_tile(m, psB))] for kb in (2, 4, 6, 8)},
            (1, 1): {kb: [(lambda m=3 + kb // 2: outproj_tile(m, psB))] for kb in (2, 4, 6, 8)},
        }

        for qc in range(NQC):
            for h in range(HL):
                if h < 2:
                    kT_h = kT01[h * DH : (h + 1) * DH, :]
                    qT_h = qT01[h * DH : (h + 1) * DH, :]
                    X_h = X01[h * DH : (h + 1) * DH, :]
                else:
                    kT_h, qT_h, X_h = kT2[:], qT2[:], X2[:]
                drip = fillers.get((qc, h), {})
                xps = psB.tile([65, QC], F32, tag="B")
                for kb in range(NKB):
                    sc = psA.tile([P, QC], F32, tag="A")
                    for j in range(QC // 512):
                        nc.tensor.matmul(
                            sc[:, j * 512 : (j + 1) * 512],
                            kT_h[:, kb * P : (kb + 1) * P],
                            qT_h[:, qc * QC + j * 512 : qc * QC + (j + 1) * 512],
                            start=True, stop=True,
                        )
                    for fn in drip.get(kb, ()):
                        fn()
                    es = sb_es.tile([P, QC], BF16, tag="es")
                    nc.scalar.activation(es[:], sc[:], EXP, scale=SCALE)
                    for j in range(QC // 512):
                        sl = slice(j * 512, (j + 1) * 512)
                        nc.tensor.matmul(
                            xps[:, sl],
                            vA[:, (h * NKB + kb) * 65 : (h * NKB + kb + 1) * 65],
                            es[:, sl],
                            start=(kb == 0), stop=(kb == NKB - 1),
                        )
                # softmax denominator -> reciprocal -> broadcast -> normalize
                dn = sb_sm.tile([1, QC], F32, tag="dn")
                nc.vector.tensor_copy(dn[:], xps[64:65, :])
                rc = sb_sm.tile([1, QC], F32, tag="rc")
                nc.vector.reciprocal_approx_fast(rc[:], dn[:])
                bcs = sb_sm.tile([DH, QC], F32, tag="bcs")
                nc.gpsimd.partition_broadcast(bcs[:], rc[:])
                nc.vector.tensor_mul(
                    X_h[:, qc * QC : (qc + 1) * QC], xps[0:DH, :], bcs[:]
                )
        # remaining out-projection tiles (alternate evac engines at the tail)
        for m in range(8, NKB):
            outproj_tile(m, psA, evac=(nc.scalar.copy if m % 2 else None))

    nc.compile()
    return nc


def _get_nc() -> bass.Bass:
    if "nc" not in _CACHE:
        _CACHE["nc"] = _build_program()
    return _CACHE["nc"]


def kernel(**inputs) -> np.ndarray:
    global LAST_RESULTS
    qs = np.asarray(inputs["query_source"], dtype=np.float32)
    kv = np.asarray(inputs["kv_source"], dtype=np.float32)
    Wq = np.asarray(inputs["Wq"], dtype=np.float32)
    Wkv = np.asarray(inputs["Wkv"], dtype=np.float32)
    Wp = np.asarray(inputs["Wproj"], dtype=np.float32)
    bp = np.asarray(inputs["bproj"], dtype=np.float32)

    nc = _get_nc()
    in_maps = []
    for c in range(8):
        b = c // 4
        c0 = (c % 4) * DL
        in_maps.append(
            {
                "qsT": np.ascontiguousarray(qs[b].T).astype(bf16),
                "kvT": np.ascontiguousarray(kv[b].T).astype(bf16),
                "wq": Wq[:, c0 : c0 + DL].astype(bf16),
                "wk": Wkv[:, c0 : c0 + DL].astype(bf16),
                "wv": Wkv[:, D + c0 : D + c0 + DL].astype(bf16),
                "wp": Wp[c0 : c0 + DL, :].astype(bf16),
            }
        )

    trace = bool(int(os.environ.get("KERNEL_TRACE", "0")))
    res = run_bass_kernel_spmd(nc, in_maps, list(range(8)), trace=trace)
    LAST_RESULTS = res

    out = np.tile(bp.astype(np.float32), (2, NQ, 1))
    for c in range(8):
        out[c // 4] += res.results[c]["out"].astype(np.float32)
    return out


# revision 12
# speedup vs baseline: 1.1045x; 1.0134x over previous
"""Multi-head attention (B=2, N=2048, D=768, H=12, Dh=64) on 8 TRN2 NeuronCores.

Sharding: head-parallel Megatron-style. Core c handles batch b=c//4 and heads
[3*(c%4), 3*(c%4)+3). Each core projects q/k/v for its 3 heads (column-sliced
Wq/Wkv), runs softmax(q k^T/8) v on-chip, and computes a partial out-projection
against its row-slice of Wproj. Host sums the 4 partials per batch + bias.

On-chip layout: sources are host-pre-transposed so projections are natural
matmuls. Scores are computed transposed (S^T: k on partitions, q free) so the
attn@v matmul consumes exp(S^T) directly as the streaming operand with
lhsT = [v | ones]; the ones column yields the softmax denominator for free.
"""

import os
import sys

sys.path.insert(0, "/opt/trn_rl_repo")

from contextlib import ExitStack

import ml_dtypes
import numpy as np

import concourse.bass as bass
import concourse.bacc as bacc
import concourse.tile as tile
from concourse import mybir
from concourse.bass_utils import run_bass_kernel_spmd

bf16 = ml_dtypes.bfloat16
F32 = mybir.dt.float32
BF16 = mybir.dt.bfloat16
EXP = mybir.ActivationFunctionType.Exp

P = 128          # partitions
NQ = 2048        # query length (per batch)
NKV = 2048       # kv length
D = 768          # model dim
DH = 64          # head dim
HL = 3           # heads per core
DL = HL * DH     # local projected dim (192)
KB = D // P      # contraction blocks for projections (6)
NKB = NKV // P   # k-index blocks (16)
QC = 1024        # q chunk for the attention inner loop
NQC = NQ // QC   # 2
SCALE = DH ** -0.5

_CACHE: dict = {}
LAST_RESULTS = None


def _build_program() -> bass.Bass:
    nc = bacc.Bacc("TRN2", target_bir_lowering=False)

    qsT = nc.dram_tensor("qsT", [D, NQ], BF16, kind="ExternalInput")
    kvT = nc.dram_tensor("kvT", [D, NKV], BF16, kind="ExternalInput")
    wq = nc.dram_tensor("wq", [D, DL], BF16, kind="ExternalInput")
    wk = nc.dram_tensor("wk", [D, DL], BF16, kind="ExternalInput")
    wv = nc.dram_tensor("wv", [D, DL], BF16, kind="ExternalInput")
    wp = nc.dram_tensor("wp", [DL, D], BF16, kind="ExternalInput")
    out = nc.dram_tensor("out", [NQ, D], BF16, kind="ExternalOutput")

    with tile.TileContext(nc) as tc, ExitStack() as ctx:
        sb_src = ctx.enter_context(tc.tile_pool(name="src", bufs=KB))
        sb_w = ctx.enter_context(tc.tile_pool(name="wts", bufs=KB))
        sb_p = ctx.enter_context(tc.tile_pool(name="persist", bufs=1))
        sb_es = ctx.enter_context(tc.tile_pool(name="es", bufs=4))
        sb_sm = ctx.enter_context(tc.tile_pool(name="small", bufs=2))
        sb_ob = ctx.enter_context(tc.tile_pool(name="outsb", bufs=3))

        # ---- DMA inputs ----
        # Spread input DMAs across engine DGE queues; whole-tensor strided
        # DMAs for the small weights. Priority: q-half-0 columns first.
        qsT_sb, kvT_sb = [], []
        for kb in range(KB):
            t_kv = sb_src.tile([P, NKV], BF16, tag="kvT")
            kvT_sb.append(t_kv)
            t_qs = sb_src.tile([P, NQ], BF16, tag="qsT")
            qsT_sb.append(t_qs)
        wkt = sb_w.tile([P, KB * DL], BF16, tag="wk")
        wvt = sb_w.tile([P, KB * DL], BF16, tag="wv")
        wqt = sb_w.tile([P, KB * DL], BF16, tag="wq")
        for t, dram in ((wkt, wk), (wvt, wv), (wqt, wq)):
            nc.scalar.dma_start(
                t[:].rearrange("p (k c) -> p k c", k=KB),
                dram[:].rearrange("(k p) c -> p k c", k=KB),
            )
        wk_sb = [wkt[:, kb * DL : (kb + 1) * DL] for kb in range(KB)]
        wv_sb = [wvt[:, kb * DL : (kb + 1) * DL] for kb in range(KB)]
        wq_sb = [wqt[:, kb * DL : (kb + 1) * DL] for kb in range(KB)]
        qs = [nc.sync, nc.gpsimd, nc.scalar]
        for kb in range(KB):
            qs[kb % 3].dma_start(kvT_sb[kb][:, 0:QC], kvT[kb * P : (kb + 1) * P, 0:QC])
        for kb in range(KB):
            qs[kb % 3].dma_start(qsT_sb[kb][:, 0:QC], qsT[kb * P : (kb + 1) * P, 0:QC])
        for kb in range(KB):
            qs[kb % 3].dma_start(kvT_sb[kb][:, QC:NKV], kvT[kb * P : (kb + 1) * P, QC:NKV])
        for kb in range(KB):
            qs[kb % 3].dma_start(qsT_sb[kb][:, QC:NQ], qsT[kb * P : (kb + 1) * P, QC:NQ])
        wp01 = sb_p.tile([P, D], BF16, tag="wp01")
        nc.scalar.dma_start(wp01[:], wp[0:P, :])
        wp2 = sb_p.tile([DH, D], BF16, tag="wp2")
        nc.scalar.dma_start(wp2[:], wp[P : P + DH, :])

        # ---- persistent intermediates ----
        qT01 = sb_p.tile([P, NQ], BF16, tag="qT01")   # q^T heads 0,1 (d on partitions)
        kT01 = sb_p.tile([P, NKV], BF16, tag="kT01")
        qT2 = sb_p.tile([DH, NQ], BF16, tag="qT2")    # q^T head 2
        kT2 = sb_p.tile([DH, NKV], BF16, tag="kT2")
        vA = sb_p.tile([P, HL * NKB * 65], BF16, tag="vA")  # per (h, kb): [v(64) | ones]
        X01 = sb_p.tile([P, NQ], BF16, tag="X01")     # normalized x^T heads 0,1
        X2 = sb_p.tile([DH, NQ], BF16, tag="X2")
        nc.vector.memset(vA[:], 1.0)  # ones columns; v evacs overwrite the rest

        # Two kernel-lifetime PSUM pools (no phase barriers):
        #   psA (2 slots x 2 banks): scores + kT01/qT01-h0 startup chains
        #   psB (2 slots x 2 banks): v-proj, attn@v accumulators, dripped fillers
        psA = ctx.enter_context(tc.tile_pool(name="psA", bufs=2, space="PSUM"))
        psB = ctx.enter_context(tc.tile_pool(name="psB", bufs=2, space="PSUM"))

        def proj01_chain(pool, w_sb, src_sb, dst, half, evac):
            """(128,1024) chain: dst[:, half] = (w block cols 0:128).T @ srcT."""
            ps = pool.tile([P, QC], F32, tag="A" if pool is psA else "B")
            for kb in range(KB):
                for j in range(QC // 512):
                    nc.tensor.matmul(
                        ps[:, j * 512 : (j + 1) * 512],
                        w_sb[kb][:, 0:P],
                        src_sb[kb][:, half * QC + j * 512 : half * QC + (j + 1) * 512],
                        start=(kb == 0),
                        stop=(kb == KB - 1),
                    )
            evac(dst[:, half * QC : (half + 1) * QC], ps[:])

        def proj2_chain(pool, half):
            """col-tiled pair: qT2 (psum 0:64) / kT2 (psum 64:128) for one q-half."""
            ps = pool.tile([P, QC], F32, tag="A" if pool is psA else "B")
            for kb in range(KB):
                for j in range(QC // 512):
                    sl = slice(j * 512, (j + 1) * 512)
                    src_sl = slice(half * QC + j * 512, half * QC + (j + 1) * 512)
                    nc.tensor.matmul(
                        ps[0:DH, sl], wq_sb[kb][:, P:DL], qsT_sb[kb][:, src_sl],
                        start=(kb == 0), stop=(kb == KB - 1),
                    )
                    nc.tensor.matmul(
                        ps[DH:P, sl], wk_sb[kb][:, P:DL], kvT_sb[kb][:, src_sl],
                        start=(kb == 0), stop=(kb == KB - 1),
                    )
            nc.vector.tensor_copy(qT2[:, half * QC : (half + 1) * QC], ps[0:DH, :])
            nc.vector.tensor_copy(kT2[:, half * QC : (half + 1) * QC], ps[DH:P, :])

        vA_view = vA[:].rearrange("p (h k c) -> p h k c", h=HL, k=NKB)

        def v_chain(m):
            ps = psB.tile([P, DL], F32, tag="B")
            for kb in range(KB):
                nc.tensor.matmul(
                    ps[:], kvT_sb[kb][:, m * P : (m + 1) * P], wv_sb[kb][:],
                    start=(kb == 0), stop=(kb == KB - 1),
                )
            nc.vector.tensor_copy(
                vA_view[:, :, m, 0:DH],
                ps[:].rearrange("p (h d) -> p h d", h=HL),
            )

        def outproj_tile(m, pool, evac=None):
            po = pool.tile([P, D], F32, tag="A" if pool is psA else "B")
            for j, n in ((0, 512), (512, 256)):
                nc.tensor.matmul(
                    po[:, j : j + n], X01[:, m * P : (m + 1) * P], wp01[:, j : j + n],
                    start=True, stop=False,
                )
                nc.tensor.matmul(
                    po[:, j : j + n], X2[:, m * P : (m + 1) * P], wp2[:, j : j + n],
                    start=False, stop=True,
                )
            ob = sb_ob.tile([P, D], BF16, tag="ob")
            (evac or nc.vector.tensor_copy)(ob[:], po[:])
            nc.sync.dma_start(out[m * P : (m + 1) * P, :], ob[:])

        # ---- startup: the minimum needed for head-0 / q-half-0 scores ----
        proj01_chain(psA, wk_sb, kvT_sb, kT01, 0, nc.scalar.copy)   # scores kb 0..7
        proj01_chain(psA, wq_sb, qsT_sb, qT01, 0, nc.scalar.copy)   # q-half 0
        v_chain(0)

        # Fillers dripped into the attention kb-loops (emitted at given kb index).
        f00 = {kb: [(lambda m=kb + 1: v_chain(m))] for kb in range(0, 15)}
        f00[3] = f00[3] + [lambda: proj01_chain(psB, wk_sb, kvT_sb, kT01, 1, nc.scalar.copy)]
        fillers = {
            (0, 0): f00,
            (0, 1): {
                4: [lambda: proj01_chain(psB, wq_sb, qsT_sb, qT01, 1, nc.vector.tensor_copy)],
                9: [lambda: proj2_chain(psB, 0)],
            },
            (0, 2): {2: [lambda: proj2_chain(psB, 1)]},
            (1, 0): {kb: [(lambda m=kb // 2 - 2: outproj_tile(m, psB))] for kb in (4, 6, 8, 10)},
            (1, 1): {kb: [(lambda m=2 + kb // 2: outproj_tile(m, psB))] for kb in (4, 6, 8, 10)},
        }

        for qc in range(NQC):
            for h in range(HL):
                if h < 2:
                    kT_h = kT01[h * DH : (h + 1) * DH, :]
                    qT_h = qT01[h * DH : (h + 1) * DH, :]
                    X_h = X01[h * DH : (h + 1) * DH, :]
                else:
                    kT_h, qT_h, X_h = kT2[:], qT2[:], X2[:]
                drip = fillers.get((qc, h), {})
                xps = psB.tile([65, QC], F32, tag="B")
                for kb in range(NKB):
                    sc = psA.tile([P, QC], F32, tag="A")
                    for j in range(QC // 512):
                        nc.tensor.matmul(
                            sc[:, j * 512 : (j + 1) * 512],
                            kT_h[:, kb * P : (kb + 1) * P],
                            qT_h[:, qc * QC + j * 512 : qc * QC + (j + 1) * 512],
                            start=True, stop=True,
                        )
                    for fn in drip.get(kb, ()):
                        fn()
                    es = sb_es.tile([P, QC], BF16, tag="es")
                    nc.scalar.activation(es[:], sc[:], EXP, scale=SCALE)
                    for j in range(QC // 512):
                        sl = slice(j * 512, (j + 1) * 512)
                        nc.tensor.matmul(
                            xps[:, sl],
                            vA[:, (h * NKB + kb) * 65 : (h * NKB + kb + 1) * 65],
                            es[:, sl],
                            start=(kb == 0), stop=(kb == NKB - 1),
                        )
                # softmax denominator -> reciprocal -> broadcast -> normalize
                dn = sb_sm.tile([1, QC], F32, tag="dn")
                nc.vector.tensor_copy(dn[:], xps[64:65, :])
                rc = sb_sm.tile([1, QC], F32, tag="rc")
                nc.vector.reciprocal_approx_fast(rc[:], dn[:])
                bcs = sb_sm.tile([DH, QC], F32, tag="bcs")
                nc.gpsimd.partition_broadcast(bcs[:], rc[:])
                nc.vector.tensor_mul(
                    X_h[:, qc * QC : (qc + 1) * QC], xps[0:DH, :], bcs[:]
                )
        # remaining out-projection tiles (alternate evac engines at the tail)
        for m in range(8, NKB):
            outproj_tile(m, psA, evac=(nc.scalar.copy if m % 2 else None))

    nc.compile()
    return nc


def _get_nc() -> bass.Bass:
    if "nc" not in _CACHE:
        _CACHE["nc"] = _build_program()
    return _CACHE["nc"]


def kernel(**inputs) -> np.ndarray:
    global LAST_RESULTS
    qs = np.asarray(inputs["query_source"], dtype=np.float32)
    kv = np.asarray(inputs["kv_source"], dtype=np.float32)
    Wq = np.asarray(inputs["Wq"], dtype=np.float32)
    Wkv = np.asarray(inputs["Wkv"], dtype=np.float32)
    Wp = np.asarray(inputs["Wproj"], dtype=np.float32)
    bp = np.asarray(inputs["bproj"], dtype=np.float32)

    nc = _get_nc()
    in_maps = []
    for c in range(8):
        b = c // 4
        c0 = (c % 4) * DL
        in_maps.append(
            {
                "qsT": np.ascontiguousarray(qs[b].T).astype(bf16),
                "kvT": np.ascontiguousarray(kv[b].T).astype(bf16),
                "wq": Wq[:, c0 : c0 + DL].astype(bf16),
                "wk": Wkv[:, c0 : c0 + DL].astype(bf16),
                "wv": Wkv[:, D + c0 : D + c0 + DL].astype(bf16),
                "wp": Wp[c0 : c0 + DL, :].astype(bf16),
            }
        )

    trace = bool(int(os.environ.get("KERNEL_TRACE", "0")))
    res = run_bass_kernel_spmd(nc, in_maps, list(range(8)), trace=trace)
    LAST_RESULTS = res

    out = np.tile(bp.astype(np.float32), (2, NQ, 1))
    for c in range(8):
        out[c // 4] += res.results[c]["out"].astype(np.float32)
    return out
